# revision 16
# baseline (speedup 1.0000x reference)
"""Causal self-attention kernel for TRN2 (8 NeuronCores, Bass/Tile).

Problem: B=8, T=1024, C=768, H=12, HD=64.
  qkv = x @ W_attn + b_attn ; causal softmax attention ; y = att_out @ W_proj + b_proj

Sharding: pure data-parallel over batch - core b computes batch element b.

v2 design (vs baseline):
  - x transposed on HOST -> xT loaded directly (no PE transposes).
  - q/k projection in fp8e4 DoubleRow (K=256 per matmul): host packs
    W_qk (x32 scale) with a quad-head column permutation so the PSUM
    output partitions are already in the [4 heads x 32 hd] layout the
    fp8 DoubleRow ST matmul wants. xT also packed fp8 (pair layout).
  - qkT stored fp8e4 (values x32); ST matmul fp8 DoubleRow
    (lhsT/rhs [32, 2, N] per head), exp scale folds the 1/1024.
  - V path, PV, proj stay bf16 (fp8 there fails the 2e-2 gate).
  - Vp head copies on GpSimd (Scalar does exp only).
  - normalization: 1/Z on DVE [1,512], broadcast via PE ones-column
    matmul (f32r), single DVE mult straight from PSUM.
  - proj result DMA'd to HBM straight from PSUM.
"""

import numpy as np

import concourse.bass as bass
import concourse.mybir as mybir
import concourse.tile as tile
from concourse import bacc
from concourse.bass_utils import run_bass_kernel_spmd

F32 = mybir.dt.float32
F32R = mybir.dt.float32r
F16 = mybir.dt.float16
BF16 = mybir.dt.bfloat16
F8 = mybir.dt.float8e4
AF = mybir.ActivationFunctionType
ALU = mybir.AluOpType
DR = mybir.MatmulPerfMode.DoubleRow

T, C, H, HD = 1024, 768, 12, 64
NCORES = 8
CC = C // 128          # 6 contraction chunks (bf16 path)
TP = T // 128          # 8 t-chunks of 128
TB = T // 512          # 2 t-blocks of 512
WSCALE = 32.0          # host scale on W_qk / b_qk (q,k carry x32)
SCALE = (1.0 / 8.0) / (WSCALE * WSCALE)   # 1/sqrt(64) / 32^2
EXP_BIAS = -8.0 * float(np.log(2.0))       # est scaled 2^-8: keeps 1/Z in fp16 range

_PROGRAM_CACHE = {}


def _qk_col_index():
    """c' column for (qk, tt, s, u): output partition u of the fp8
    projection holds head (3*tt + u//32), hd dim s*32 + u%32.
    Triads of 3 heads per 96 partitions (AP base must be 0/32/64)."""
    idx = np.zeros((2, 4, 2, 96), np.int64)
    for qk in range(2):
        for tt in range(4):
            for s in range(2):
                u = np.arange(96)
                idx[qk, tt, s] = qk * C + (3 * tt + u // 32) * HD + s * 32 + (u % 32)
    return idx


def build_program():
    nc = bacc.Bacc("TRN2", target_bir_lowering=False, debug=False)

    xt_d = nc.dram_tensor("xT", [C, T], BF16, kind="ExternalInput").ap()
    x8_d = nc.dram_tensor("x8", [128, 3 * 2 * T], F8, kind="ExternalInput").ap()
    w8_d = nc.dram_tensor("w8", [128, 3 * 2 * 16 * 96], F8, kind="ExternalInput").ap()
    wv_d = nc.dram_tensor("Wv", [C, C], BF16, kind="ExternalInput").ap()
    wp_d = nc.dram_tensor("W_proj", [C, C], BF16, kind="ExternalInput").ap()
    bqk_d = nc.dram_tensor("bqk", [128, 16], F32, kind="ExternalInput").ap()
    bv_d = nc.dram_tensor("bv", [1, C], BF16, kind="ExternalInput").ap()
    bp_d = nc.dram_tensor("b_proj", [1, C], BF16, kind="ExternalInput").ap()
    y_d = nc.dram_tensor("y", [T, C], F32, kind="ExternalOutput").ap()

    with tile.TileContext(nc) as tc:
        _emit(nc, tc, xt_d, x8_d, w8_d, wv_d, wp_d, bqk_d, bv_d, bp_d, y_d)
    nc.compile()
    return nc


def _emit(nc, tc, xt_d, x8_d, w8_d, wv_d, wp_d, bqk_d, bv_d, bp_d, y_d):
    from contextlib import ExitStack

    ctx = ExitStack()
    with ctx:
        const_pool = ctx.enter_context(tc.tile_pool(name="consts", bufs=1))
        # ps_work: [128,1024] ST tiles (2 banks each, 2 bufs = 4 banks)
        # ps_acc: 1-bank accumulators (qk/v/y) + zb broadcast tiles
        # po: PV accumulators (tag "ot")
        ps_work = ctx.enter_context(tc.tile_pool(name="ps_work", bufs=2, space="PSUM"))
        ps_acc = ctx.enter_context(tc.tile_pool(name="ps_acc", bufs=2, space="PSUM"))

        # ---- input DMAs in consumption order ---------------------------
        in_pool = ctx.enter_context(tc.tile_pool(name="inputs", bufs=1, side="right"))
        x8 = in_pool.tile([128, 3, 2, T], F8, name="x8")
        nc.sync.dma_start(x8[:].rearrange("p j s t -> p (j s t)"), x8_d[:, :])
        w8 = in_pool.tile([128, 3, 2, 16, 96], F8, name="w8")
        nc.sync.dma_start(w8[:].rearrange("p j s b u -> p (j s b u)"), w8_d[:, :])
        bqk = const_pool.tile([128, 16], F32, name="bqk")
        nc.sync.dma_start(bqk[:], bqk_d[:, :])

        xT = []
        for cc in range(CC):
            t_ = in_pool.tile([128, T], BF16, name=f"xT_{cc}", tag=f"xT{cc}")
            nc.sync.dma_start(t_[:], xt_d[cc * 128 : (cc + 1) * 128, :])
            xT.append(t_)
        Wv = []
        for cc in range(CC):
            t_ = in_pool.tile([128, C], BF16, name=f"Wv_{cc}", tag=f"Wv{cc}")
            nc.sync.dma_start(t_[:], wv_d[cc * 128 : (cc + 1) * 128, :])
            Wv.append(t_)
        bv_sb = const_pool.tile([1, C], BF16, name="bv_sb")
        nc.sync.dma_start(bv_sb[:], bv_d[:, :])
        bp_sb = const_pool.tile([1, C], BF16, name="bp_sb")
        nc.sync.dma_start(bp_sb[:], bp_d[:, :])
        wp_pool = ctx.enter_context(tc.tile_pool(name="wp", bufs=1))
        Wp = []
        for cc in range(CC):
            w_t = wp_pool.tile([128, C], BF16, name=f"Wp_{cc}", tag=f"Wp{cc}")
            nc.sync.dma_start(w_t[:], wp_d[cc * 128 : (cc + 1) * 128, :])
            Wp.append(w_t)

        # ---- constants -------------------------------------------------
        # tri[j, i] = 1.0 if j <= i else 0.0 (keep lower-causal in [j,i])
        tri_f32 = const_pool.tile([128, 128], F32, name="tri_f32")
        nc.gpsimd.memset(tri_f32[:], 1.0)
        nc.gpsimd.affine_select(
            out=tri_f32[:], in_=tri_f32[:], compare_op=ALU.is_ge, fill=0.0,
            base=0, pattern=[[1, 128]], channel_multiplier=-1,
        )
        tri = const_pool.tile([128, 128], BF16, name="tri")
        nc.vector.tensor_copy(tri[:], tri_f32[:])
        ones32 = const_pool.tile([128, 16], F32, name="ones32")
        nc.gpsimd.memset(ones32[:], 1.0)
        ones_row = const_pool.tile([1, 512], BF16, name="ones_row")
        nc.gpsimd.memset(ones_row[:], 1.0)
        ones_r = const_pool.tile([1, 64], F16, name="ones_r")
        nc.gpsimd.memset(ones_r[:], 1.0)
        expb = const_pool.tile([128, 1], F32, name="expb")
        nc.gpsimd.memset(expb[:], EXP_BIAS)

        # warm the exp table set early (hidden under input DMA)
        expwarm = const_pool.tile([1, 1], F32, name="expwarm")
        nc.scalar.activation(expwarm[:], ones32[0:1, 0:1], AF.Exp)

        # ---- qk projection (fp8 DoubleRow, 3 heads per 96-part triad) --
        qk_pool = ctx.enter_context(tc.tile_pool(name="qk8", bufs=1))
        QK8 = []
        for tt in range(4):
            t_ = qk_pool.tile([96, 2, 2, T], F8, name=f"QK8_{tt}", tag=f"QK8{tt}")
            QK8.append(t_)

        def qk_triad(tt):
            for tb in range(TB):
                for qk in range(2):
                    for s in range(2):
                        blk = tt * 4 + qk * 2 + s
                        pq = ps_acc.tile([96, 512], F32, name=f"ps_qk_{blk}_{tb}", tag="acc")
                        for j in range(3):
                            nc.tensor.matmul(
                                pq[:],
                                w8[:, j, :, blk, :],
                                x8[:, j, :, tb * 512 : (tb + 1) * 512],
                                start=(j == 0),
                                stop=(j == 2),
                                perf_mode=DR,
                            )
                        nc.vector.tensor_scalar_add(
                            QK8[tt][:, qk, s, tb * 512 : (tb + 1) * 512],
                            pq[:],
                            bqk[0:96, blk : blk + 1],
                        )

        # ---- V path (bf16) --------------------------------------------
        vp_pool = ctx.enter_context(tc.tile_pool(name="vp", bufs=1))
        Vp = []
        for tp in range(TP):
            t_ = vp_pool.tile([128, H * 65], BF16, name=f"Vp_{tp}", tag=f"Vp{tp}")
            Vp.append(t_)
            nc.vector.tensor_copy(
                t_.rearrange("p (h e) -> p h e", e=65)[:, :, 64:65],
                ones32[:, 0:H].rearrange("p (h e) -> p h e", e=1),
            )

        def v_chunk(tp):
            for vc in range(2):  # v cols [384*vc : 384*(vc+1)] of Wv
                pv = ps_acc.tile([128, 384], F32, name=f"ps_v_{vc}_{tp}", tag="acc")
                for cc in range(CC):
                    nc.tensor.matmul(
                        pv[:],
                        xT[cc][:, tp * 128 : (tp + 1) * 128],
                        Wv[cc][:, vc * 384 : (vc + 1) * 384],
                        start=(cc == 0),
                        stop=False,
                    )
                nc.tensor.matmul(
                    pv[:],
                    ones_row[:, 0:128],
                    bv_sb[:, vc * 384 : (vc + 1) * 384],
                    start=False,
                    stop=True,
                )
                # one strided copy: 6 heads at once, skipping the ones column
                nc.vector.tensor_copy(
                    Vp[tp].rearrange("p (h e) -> p h e", e=65)[
                        :, 6 * vc : 6 * vc + 6, 0:64
                    ],
                    pv[:].rearrange("p (h e) -> p h e", e=64),
                )

        # ---- attention -------------------------------------------------
        atn_pool = ctx.enter_context(tc.tile_pool(name="atn", bufs=1))
        ATn = []
        for cp in range(CC):
            t_ = atn_pool.tile([128, T], BF16, name=f"ATn_{cp}", tag=f"ATn{cp}")
            ATn.append(t_)

        est_pool = ctx.enter_context(tc.tile_pool(name="est", bufs=8))
        nrm_pool = ctx.enter_context(tc.tile_pool(name="nrm", bufs=4))

        def attention(hp, ib):
            po = {}
            for s in range(2):  # head 2*hp + s
                po[s] = ps_acc.tile([65, 512], F32, name=f"ps_ot_{hp}_{ib}_{s}", tag="ot", bufs=2)
            njc = 4 * (ib + 1)
            for jc in range(njc):
                r = jc - 4 * ib
                col0 = max(r, 0) * 128
                # merged pair tile: head A in cols [0:512], head B in [512:1024]
                pst = ps_work.tile([128, 1024], F32, name=f"ps_st_{hp}_{ib}_{jc}", tag="ps")
                for s in range(2):
                    h = 2 * hp + s
                    tt, p0 = h // 3, 32 * (h % 3)
                    nc.tensor.matmul(
                        pst[:, 512 * s + col0 : 512 * s + 512],
                        QK8[tt][p0 : p0 + 32, 1, :, jc * 128 : (jc + 1) * 128],
                        QK8[tt][p0 : p0 + 32, 0, :, ib * 512 + col0 : (ib + 1) * 512],
                        start=True,
                        stop=True,
                        perf_mode=DR,
                    )
                est = est_pool.tile([128, 1024], BF16, name=f"est_{hp}_{ib}_{jc}", tag="est")
                nc.scalar.activation(
                    est.rearrange("p (a f) -> p a f", a=2)[:, :, col0:512],
                    pst.rearrange("p (a f) -> p a f", a=2)[:, :, col0:512],
                    AF.Exp,
                    scale=SCALE,
                    bias=expb[:],
                )
                if r >= 0:
                    for s in range(2):
                        # mask the diagonal 128x128 sub-block (multiplicative)
                        nc.vector.tensor_tensor(
                            est[:, 512 * s + col0 : 512 * s + col0 + 128],
                            est[:, 512 * s + col0 : 512 * s + col0 + 128],
                            tri[:],
                            op=ALU.mult,
                        )
                for s in range(2):
                    h = 2 * hp + s
                    nc.tensor.matmul(
                        po[s][:, col0:512],
                        Vp[jc][:, h * 65 : h * 65 + 65],
                        est[:, 512 * s + col0 : 512 * s + 512],
                        start=(jc == 0),
                        stop=(jc == njc - 1),
                    )
            # normalization: ATn rows = OT'[0:64] / Z  (Z = row 64).
            # 1/Z on DVE [1,512]; broadcast across partitions with a PE
            # ones-column matmul (f32r); one DVE mult straight from PSUM.
            for s in range(2):
                otu = nrm_pool.tile([65, 512], F32, name=f"otu_{hp}_{ib}_{s}", tag="otu")
                nc.vector.tensor_copy(otu[:], po[s][:, :])
                zr = nrm_pool.tile([1, 512], F16, name=f"zr_{hp}_{ib}_{s}", tag="zr")
                with nc.allow_low_precision(reason="1/Z in fp16: 2^-11 rel err scales all weights of a query equally"):
                    nc.vector.reciprocal(zr[:], otu[64:65, :])
                zb = ps_acc.tile([64, 512], F32, name=f"zb_{hp}_{ib}_{s}", tag="acc")
                nc.tensor.matmul(zb[:], ones_r[:, :], zr[:], start=True, stop=True)
                nc.vector.tensor_tensor(
                    ATn[hp][64 * s : 64 * s + 64, ib * 512 : (ib + 1) * 512],
                    otu[0:64, :],
                    zb[:],
                    op=ALU.mult,
                )

        y_pool = ctx.enter_context(tc.tile_pool(name="ysb", bufs=2))

        def proj(tp):
            y_sb = y_pool.tile([128, C], F32, name=f"y_sb_{tp}", tag="y_sb")
            for oc in range(2):
                py = ps_acc.tile([128, 384], F32, name=f"ps_y_{tp}_{oc}", tag="acc")
                for cp in range(CC):
                    nc.tensor.matmul(
                        py[:],
                        ATn[cp][:, tp * 128 : (tp + 1) * 128],
                        Wp[cp][:, oc * 384 : (oc + 1) * 384],
                        start=(cp == 0),
                        stop=False,
                    )
                nc.tensor.matmul(
                    py[:],
                    ones_row[:, 0:128],
                    bp_sb[:, oc * 384 : (oc + 1) * 384],
                    start=False,
                    stop=True,
                )
                nc.vector.tensor_copy(y_sb[:, oc * 384 : (oc + 1) * 384], py[:])
            nc.sync.dma_start(y_d[tp * 128 : (tp + 1) * 128, :], y_sb[:])

        # Emission schedule: qk quads first (their fp8 inputs land first),
        # then V chunks; attention ib=0 as soon as deps exist; ib=1
        # interleaved with proj.
        qk_triad(0)
        qk_triad(1)
        for tp in range(4):
            v_chunk(tp)
        attention(0, 0)
        qk_triad(2)
        attention(1, 0)
        qk_triad(3)
        for tp in range(4, 8):
            v_chunk(tp)
        for hp in range(2, 6):
            attention(hp, 0)
        attention(0, 1)
        attention(1, 1)
        proj(0)
        attention(2, 1)
        proj(1)
        attention(3, 1)
        proj(2)
        attention(4, 1)
        proj(3)
        attention(5, 1)
        for tp in range(4, 8):
            proj(tp)


def kernel(x, W_attn, b_attn, W_proj, b_proj, _trace=False, _trace_kwargs=None):
    import ml_dtypes

    bf16 = ml_dtypes.bfloat16
    f8 = ml_dtypes.float8_e4m3

    x = np.asarray(x, np.float32)
    W_attn = np.asarray(W_attn, np.float32)
    b_attn = np.asarray(b_attn, np.float32)
    W_proj = np.ascontiguousarray(np.asarray(W_proj).astype(bf16))
    b_proj = np.ascontiguousarray(np.asarray(b_proj).astype(bf16)).reshape(1, C)

    # W_qk packed for fp8 DoubleRow: [p, j, s, blk, u]
    idx = _qk_col_index()  # [qk, tt, s, u]
    wqk = (WSCALE * W_attn[:, : 2 * C]).reshape(3, 2, 128, 2 * C)  # [j, js, p, c']
    w8 = np.zeros((128, 3, 2, 16, 96), np.float32)
    bqk = np.zeros((128, 16), np.float32)
    for qk in range(2):
        for tt in range(4):
            for s in range(2):
                blk = tt * 4 + qk * 2 + s
                w8[:, :, :, blk, :] = wqk[:, :, :, idx[qk, tt, s]].transpose(2, 0, 1, 3)
                bqk[0:96, blk] = WSCALE * b_attn[idx[qk, tt, s]]
    w8 = np.ascontiguousarray(w8.reshape(128, -1).astype(f8))
    bqk = np.ascontiguousarray(bqk)
    wv = np.ascontiguousarray(W_attn[:, 2 * C :].astype(bf16))
    bv = np.ascontiguousarray(b_attn[2 * C :].astype(bf16)).reshape(1, C)

    if "prog" not in _PROGRAM_CACHE:
        _PROGRAM_CACHE["prog"] = build_program()
    nc = _PROGRAM_CACHE["prog"]

    in_maps = []
    for b in range(NCORES):
        xt = np.ascontiguousarray(x[b].T)  # [C, T] f32
        x8 = np.ascontiguousarray(
            xt.reshape(3, 2, 128, T).transpose(2, 0, 1, 3).reshape(128, -1).astype(f8)
        )
        in_maps.append(
            {
                "xT": xt.astype(bf16),
                "x8": x8,
                "w8": w8,
                "Wv": wv,
                "W_proj": W_proj,
                "bqk": bqk,
                "bv": bv,
                "b_proj": b_proj,
            }
        )
    res = run_bass_kernel_spmd(
        nc,
        in_maps,
        core_ids=list(range(NCORES)),
        trace=_trace,
        **(_trace_kwargs or {}),
    )
    out = np.stack([res.results[b]["y"] for b in range(NCORES)], axis=0)
    if _trace:
        return out, res
    return out


if __name__ == "__main__":
    rng = np.random.default_rng(0)
    x = rng.standard_normal((NCORES, T, C)).astype(np.float32)
    W_attn = (rng.standard_normal((C, 3 * C)) * 0.02).astype(np.float32)
    b_attn = np.zeros(3 * C, np.float32)
    W_proj = (rng.standard_normal((C, C)) * 0.02).astype(np.float32)
    b_proj = np.zeros(C, np.float32)
    y = kernel(x=x, W_attn=W_attn, b_attn=b_attn, W_proj=W_proj, b_proj=b_proj)
    print("out", y.shape, y.dtype, np.abs(y).max())


# revision 18
# speedup vs baseline: 1.3430x; 1.3430x over previous
"""Causal self-attention kernel for TRN2 (8 NeuronCores, Bass/Tile).

Problem: B=8, T=1024, C=768, H=12, HD=64.
  qkv = x @ W_attn + b_attn ; causal softmax attention ; y = att_out @ W_proj + b_proj

Sharding: pure data-parallel over batch - core b computes batch element b.

v3 design (all matmuls bf16):
  - x transposed on HOST -> xT loaded directly (no PE transposes).
  - W_qk host-permuted pair-major (256-col blocks per head pair) so the
    first attention pair's weights arrive first; per-(hp,cc) DMAs let
    the qk chains stream right behind the DMA.
  - b_attn qk part host-packed as per-partition columns [128, 12].
  - Vp head copies: one strided DVE copy per (tp, vc) (6 heads at once).
  - Fine-grained interleave: qk/v/proj matmul chains are chopped into
    small units and pumped into the PE queue between attention jc
    iterations, so the PE never stalls waiting for exp (Scalar) and
    stays at the fast p-state.
  - proj contracts ATn[5] last so only the last norm gates the tail.
"""

import numpy as np

import concourse.bass as bass
import concourse.mybir as mybir
import concourse.tile as tile
from concourse import bacc
from concourse.bass_utils import run_bass_kernel_spmd

F32 = mybir.dt.float32
BF16 = mybir.dt.bfloat16
AF = mybir.ActivationFunctionType
ALU = mybir.AluOpType

T, C, H, HD = 1024, 768, 12, 64
NCORES = 8
CC = C // 128          # 6 contraction chunks
TP = T // 128          # 8 t-chunks of 128
TB = T // 512          # 2 t-blocks of 512
NP = 6                 # head pairs
SCALE = 1.0 / 8.0      # 1/sqrt(64)

_PROGRAM_CACHE = {}


def build_program():
    nc = bacc.Bacc("TRN2", target_bir_lowering=False, debug=False)

    xt_d = nc.dram_tensor("xT", [C, T], BF16, kind="ExternalInput").ap()
    wqk_d = nc.dram_tensor("wqk", [C, 2 * C], BF16, kind="ExternalInput").ap()
    wv_d = nc.dram_tensor("Wv", [C, C], BF16, kind="ExternalInput").ap()
    wp_d = nc.dram_tensor("W_proj", [C, C], BF16, kind="ExternalInput").ap()
    bqk_d = nc.dram_tensor("bqk", [128, 12], F32, kind="ExternalInput").ap()
    bv_d = nc.dram_tensor("bv", [1, C], BF16, kind="ExternalInput").ap()
    bp_d = nc.dram_tensor("b_proj", [1, C], BF16, kind="ExternalInput").ap()
    y_d = nc.dram_tensor("y", [T, C], F32, kind="ExternalOutput").ap()

    with tile.TileContext(nc) as tc:
        _emit(nc, tc, xt_d, wqk_d, wv_d, wp_d, bqk_d, bv_d, bp_d, y_d)
    nc.compile()
    return nc


def _emit(nc, tc, xt_d, wqk_d, wv_d, wp_d, bqk_d, bv_d, bp_d, y_d):
    from contextlib import ExitStack
    from collections import deque

    ctx = ExitStack()
    with ctx:
        const_pool = ctx.enter_context(tc.tile_pool(name="consts", bufs=1))
        # ps_work: [128,1024] ST tiles (2 banks x 2 bufs); ps_acc: 1-bank
        # accumulators (qk/v/y); po: PV accumulators (tag "ot").
        ps_work = ctx.enter_context(tc.tile_pool(name="ps_work", bufs=2, space="PSUM"))
        ps_acc = ctx.enter_context(tc.tile_pool(name="ps_acc", bufs=2, space="PSUM"))

        # ---- input DMAs, interleaved in consumption order ---------------
        in_pool = ctx.enter_context(tc.tile_pool(name="inputs", bufs=1, side="right"))
        xT, Wqk, Wv = [], [], []
        for cc in range(CC):
            xT.append(in_pool.tile([128, T], BF16, name=f"xT_{cc}", tag=f"xT{cc}"))
            Wqk.append(in_pool.tile([128, 2 * C], BF16, name=f"Wqk_{cc}", tag=f"Wqk{cc}"))
            Wv.append(in_pool.tile([128, C], BF16, name=f"Wv_{cc}", tag=f"Wv{cc}"))
        for cc in range(CC):
            nc.sync.dma_start(xT[cc][:], xt_d[cc * 128 : (cc + 1) * 128, :])
            nc.sync.dma_start(
                Wqk[cc][:, 0:256], wqk_d[cc * 128 : (cc + 1) * 128, 0:256]
            )
        for cc in range(CC):
            nc.sync.dma_start(
                Wqk[cc][:, 256:512], wqk_d[cc * 128 : (cc + 1) * 128, 256:512]
            )
        bqk = const_pool.tile([128, 12], F32, name="bqk")
        nc.sync.dma_start(bqk[:], bqk_d[:, :])
        for cc in range(CC):
            nc.sync.dma_start(Wv[cc][:], wv_d[cc * 128 : (cc + 1) * 128, :])
        bv_sb = const_pool.tile([1, C], BF16, name="bv_sb")
        nc.sync.dma_start(bv_sb[:], bv_d[:, :])
        bp_sb = const_pool.tile([1, C], BF16, name="bp_sb")
        nc.sync.dma_start(bp_sb[:], bp_d[:, :])
        for cc in range(CC):
            nc.sync.dma_start(
                Wqk[cc][:, 512:1536], wqk_d[cc * 128 : (cc + 1) * 128, 512:1536]
            )
        wp_pool = ctx.enter_context(tc.tile_pool(name="wp", bufs=1))
        Wp = []
        for cc in range(CC):
            w_t = wp_pool.tile([128, C], BF16, name=f"Wp_{cc}", tag=f"Wp{cc}")
            nc.sync.dma_start(w_t[:], wp_d[cc * 128 : (cc + 1) * 128, :])
            Wp.append(w_t)

        # ---- constants -------------------------------------------------
        tri_f32 = const_pool.tile([128, 128], F32, name="tri_f32")
        nc.gpsimd.memset(tri_f32[:], 1.0)
        nc.gpsimd.affine_select(
            out=tri_f32[:], in_=tri_f32[:], compare_op=ALU.is_ge, fill=0.0,
            base=0, pattern=[[1, 128]], channel_multiplier=-1,
        )
        tri = const_pool.tile([128, 128], BF16, name="tri")
        nc.vector.tensor_copy(tri[:], tri_f32[:])
        ones32 = const_pool.tile([128, 16], F32, name="ones32")
        nc.gpsimd.memset(ones32[:], 1.0)
        ones_row = const_pool.tile([1, 512], BF16, name="ones_row")
        nc.gpsimd.memset(ones_row[:], 1.0)

        expwarm = const_pool.tile([1, 1], F32, name="expwarm")
        nc.scalar.activation(expwarm[:], ones32[0:1, 0:1], AF.Exp)

        # ---- qk projection (bf16) --------------------------------------
        qkt_pool = ctx.enter_context(tc.tile_pool(name="qkt", bufs=1))
        qkT = {}
        for hp in range(NP):
            for qk in range(2):
                qkT[(hp, qk)] = qkt_pool.tile(
                    [128, T], BF16, name=f"qkT_{hp}_{qk}", tag=f"qkT{hp}{qk}"
                )

        def qk_chain_units(hp, qk, tb):
            """Two units: a [128,512] projection chain split in halves."""
            col = hp * 256 + qk * 128
            pq = ps_acc.tile([128, 512], F32, name=f"ps_qk_{hp}_{qk}_{tb}", tag="acc")

            def u1():
                for cc in range(3):
                    nc.tensor.matmul(
                        pq[:],
                        Wqk[cc][:, col : col + 128],
                        xT[cc][:, tb * 512 : (tb + 1) * 512],
                        start=(cc == 0),
                        stop=False,
                    )

            def u2():
                for cc in range(3, CC):
                    nc.tensor.matmul(
                        pq[:],
                        Wqk[cc][:, col : col + 128],
                        xT[cc][:, tb * 512 : (tb + 1) * 512],
                        start=False,
                        stop=(cc == CC - 1),
                    )
                nc.vector.tensor_scalar_add(
                    qkT[(hp, qk)][:, tb * 512 : (tb + 1) * 512],
                    pq[:],
                    bqk[:, hp * 2 + qk : hp * 2 + qk + 1],
                )

            return [u1, u2]

        # ---- V path ----------------------------------------------------
        vp_pool = ctx.enter_context(tc.tile_pool(name="vp", bufs=1))
        Vp = []
        for tp in range(TP):
            t_ = vp_pool.tile([128, H * 65], BF16, name=f"Vp_{tp}", tag=f"Vp{tp}")
            Vp.append(t_)
            nc.vector.tensor_copy(
                t_.rearrange("p (h e) -> p h e", e=65)[:, :, 64:65],
                ones32[:, 0:H].rearrange("p (h e) -> p h e", e=1),
            )

        def v_chain_units(tp, vc):
            pv = ps_acc.tile([128, 384], F32, name=f"ps_v_{vc}_{tp}", tag="acc")

            def u1():
                for cc in range(3):
                    nc.tensor.matmul(
                        pv[:],
                        xT[cc][:, tp * 128 : (tp + 1) * 128],
                        Wv[cc][:, vc * 384 : (vc + 1) * 384],
                        start=(cc == 0),
                        stop=False,
                    )

            def u2():
                for cc in range(3, CC):
                    nc.tensor.matmul(
                        pv[:],
                        xT[cc][:, tp * 128 : (tp + 1) * 128],
                        Wv[cc][:, vc * 384 : (vc + 1) * 384],
                        start=False,
                        stop=False,
                    )
                nc.tensor.matmul(
                    pv[:],
                    ones_row[:, 0:128],
                    bv_sb[:, vc * 384 : (vc + 1) * 384],
                    start=False,
                    stop=True,
                )
                nc.vector.tensor_copy(
                    Vp[tp].rearrange("p (h e) -> p h e", e=65)[
                        :, 6 * vc : 6 * vc + 6, 0:64
                    ],
                    pv[:].rearrange("p (h e) -> p h e", e=64),
                )

            return [u1, u2]

        # ---- attention -------------------------------------------------
        atn_pool = ctx.enter_context(tc.tile_pool(name="atn", bufs=1))
        ATn = []
        for cp in range(CC):
            t_ = atn_pool.tile([128, T], BF16, name=f"ATn_{cp}", tag=f"ATn{cp}")
            ATn.append(t_)

        est_pool = ctx.enter_context(tc.tile_pool(name="est", bufs=8))
        nrm_pool = ctx.enter_context(tc.tile_pool(name="nrm", bufs=4))
        y_pool = ctx.enter_context(tc.tile_pool(name="ysb", bufs=2))

        fillers = deque()

        def pump(n):
            for _ in range(n):
                if fillers:
                    fillers.popleft()()

        def attention(hp, ib, npump):
            qt = qkT[(hp, 0)]
            kt = qkT[(hp, 1)]
            po = {}
            for s in range(2):  # head 2*hp + s
                po[s] = ps_acc.tile(
                    [65, 512], F32, name=f"ps_ot_{hp}_{ib}_{s}", tag="ot", bufs=2
                )
            njc = 4 * (ib + 1)
            for jc in range(njc):
                r = jc - 4 * ib
                col0 = max(r, 0) * 128
                pst = ps_work.tile([128, 1024], F32, name=f"ps_st_{hp}_{ib}_{jc}", tag="ps")
                for s in range(2):
                    r0 = 64 * s
                    nc.tensor.matmul(
                        pst[:, 512 * s + col0 : 512 * s + 512],
                        kt[r0 : r0 + 64, jc * 128 : (jc + 1) * 128],
                        qt[r0 : r0 + 64, ib * 512 + col0 : (ib + 1) * 512],
                        start=True,
                        stop=True,
                    )
                est = est_pool.tile([128, 1024], BF16, name=f"est_{hp}_{ib}_{jc}", tag="est")
                nc.scalar.activation(
                    est.rearrange("p (a f) -> p a f", a=2)[:, :, col0:512],
                    pst.rearrange("p (a f) -> p a f", a=2)[:, :, col0:512],
                    AF.Exp,
                    scale=SCALE,
                )
                if r >= 0:
                    for s in range(2):
                        nc.vector.tensor_tensor(
                            est[:, 512 * s + col0 : 512 * s + col0 + 128],
                            est[:, 512 * s + col0 : 512 * s + col0 + 128],
                            tri[:],
                            op=ALU.mult,
                        )
                pump(npump)
                for s in range(2):
                    h = 2 * hp + s
                    nc.tensor.matmul(
                        po[s][:, col0:512],
                        Vp[jc][:, h * 65 : h * 65 + 65],
                        est[:, 512 * s + col0 : 512 * s + 512],
                        start=(jc == 0),
                        stop=(jc == njc - 1),
                    )
            # normalization (scatter -> [128,4] recip -> gather -> bcast)
            for s in range(2):
                otu = nrm_pool.tile([65, 512], F32, name=f"otu_{hp}_{ib}_{s}", tag="otu")
                nc.vector.tensor_copy(otu[:], po[s][:, :])
                zs = nrm_pool.tile([128, 4], F32, name=f"zs_{hp}_{ib}_{s}", tag="zs")
                nc.gpsimd.dma_start(zs[:], otu[64:65, :])
                zr = nrm_pool.tile([128, 4], F32, name=f"zr_{hp}_{ib}_{s}", tag="zr")
                nc.vector.reciprocal(zr[:], zs[:])
                zinv = nrm_pool.tile([1, 512], F32, name=f"zinv_{hp}_{ib}_{s}", tag="zinv")
                nc.gpsimd.dma_start(zinv[:], zr[:])
                zb = nrm_pool.tile([64, 512], F32, name=f"zb_{hp}_{ib}_{s}", tag="zb")
                nc.gpsimd.partition_broadcast(zb[:], zinv[:])
                pump(1)
                nc.vector.tensor_tensor(
                    ATn[hp][64 * s : 64 * s + 64, ib * 512 : (ib + 1) * 512],
                    otu[0:64, :],
                    zb[:],
                    op=ALU.mult,
                )

        def proj_units(tp):
            y_sb = y_pool.tile([128, C], F32, name=f"y_sb_{tp}", tag="y_sb")
            units = []
            for oc in range(2):
                py = ps_acc.tile([128, 384], F32, name=f"ps_y_{tp}_{oc}", tag="acc")

                def u1(py=py, oc=oc):
                    for cp in range(3):
                        nc.tensor.matmul(
                            py[:],
                            ATn[cp][:, tp * 128 : (tp + 1) * 128],
                            Wp[cp][:, oc * 384 : (oc + 1) * 384],
                            start=(cp == 0),
                            stop=False,
                        )

                def u2(py=py, oc=oc, last=(oc == 1)):
                    for cp in range(3, CC):
                        nc.tensor.matmul(
                            py[:],
                            ATn[cp][:, tp * 128 : (tp + 1) * 128],
                            Wp[cp][:, oc * 384 : (oc + 1) * 384],
                            start=False,
                            stop=False,
                        )
                    nc.tensor.matmul(
                        py[:],
                        ones_row[:, 0:128],
                        bp_sb[:, oc * 384 : (oc + 1) * 384],
                        start=False,
                        stop=True,
                    )
                    nc.vector.tensor_copy(y_sb[:, oc * 384 : (oc + 1) * 384], py[:])
                    if last:
                        nc.sync.dma_start(y_d[tp * 128 : (tp + 1) * 128, :], y_sb[:])

                units += [u1, u2]
            return units

        # ---- emission schedule -----------------------------------------
        # Prologue: first two pairs + first four Vp chunks inline (these
        # cover the input DMA); the rest become fillers pumped into the
        # attention loop.
        for tb in range(TB):
            for u in qk_chain_units(0, 0, tb) + qk_chain_units(0, 1, tb):
                u()
        for tb in range(TB):
            for u in qk_chain_units(1, 0, tb) + qk_chain_units(1, 1, tb):
                u()
        for tp in range(4):
            for vc in range(2):
                for u in v_chain_units(tp, vc):
                    u()

        for hp in range(2, NP):
            for qk in range(2):
                for tb in range(TB):
                    fillers.extend(qk_chain_units(hp, qk, tb))
            # interleave remaining v chunks (tp 4..7) between pair chains
            for vc in range(2):
                fillers.extend(v_chain_units(2 + hp, vc))

        attention(0, 0, npump=3)
        attention(1, 0, npump=3)
        attention(2, 0, npump=3)
        attention(3, 0, npump=3)
        attention(4, 0, npump=2)
        attention(5, 0, npump=2)
        for tp in range(4):
            fillers.extend(proj_units(tp))
        attention(0, 1, npump=2)
        attention(1, 1, npump=2)
        attention(2, 1, npump=2)
        attention(3, 1, npump=1)
        attention(4, 1, npump=1)
        attention(5, 1, npump=1)
        pump(len(fillers))
        for tp in range(4, 8):
            for u in proj_units(tp):
                u()


def kernel(x, W_attn, b_attn, W_proj, b_proj, _trace=False, _trace_kwargs=None):
    import ml_dtypes

    bf16 = ml_dtypes.bfloat16

    x = np.asarray(x, np.float32)
    W_attn = np.asarray(W_attn, np.float32)
    b_attn = np.asarray(b_attn, np.float32)
    W_proj = np.ascontiguousarray(np.asarray(W_proj).astype(bf16))
    b_proj = np.ascontiguousarray(np.asarray(b_proj).astype(bf16)).reshape(1, C)

    # W_qk pair-major: col block hp*256 holds [q cols of pair hp | k cols]
    wqk = np.zeros((C, 2 * C), np.float32)
    bqk = np.zeros((128, 12), np.float32)
    for hp in range(NP):
        for qk in range(2):
            src = qk * C + hp * 128
            wqk[:, hp * 256 + qk * 128 : hp * 256 + qk * 128 + 128] = W_attn[
                :, src : src + 128
            ]
            bqk[:, hp * 2 + qk] = b_attn[src : src + 128]
    wqk = np.ascontiguousarray(wqk.astype(bf16))
    bqk = np.ascontiguousarray(bqk)
    wv = np.ascontiguousarray(W_attn[:, 2 * C :].astype(bf16))
    bv = np.ascontiguousarray(b_attn[2 * C :].astype(bf16)).reshape(1, C)

    if "prog" not in _PROGRAM_CACHE:
        _PROGRAM_CACHE["prog"] = build_program()
    nc = _PROGRAM_CACHE["prog"]

    in_maps = []
    for b in range(NCORES):
        in_maps.append(
            {
                "xT": np.ascontiguousarray(x[b].T.astype(bf16)),
                "wqk": wqk,
                "Wv": wv,
                "W_proj": W_proj,
                "bqk": bqk,
                "bv": bv,
                "b_proj": b_proj,
            }
        )
    res = run_bass_kernel_spmd(
        nc,
        in_maps,
        core_ids=list(range(NCORES)),
        trace=_trace,
        **(_trace_kwargs or {}),
    )
    out = np.stack([res.results[b]["y"] for b in range(NCORES)], axis=0)
    if _trace:
        return out, res
    return out


if __name__ == "__main__":
    rng = np.random.default_rng(0)
    x = rng.standard_normal((NCORES, T, C)).astype(np.float32)
    W_attn = (rng.standard_normal((C, 3 * C)) * 0.02).astype(np.float32)
    b_attn = np.zeros(3 * C, np.float32)
    W_proj = (rng.standard_normal((C, C)) * 0.02).astype(np.float32)
    b_proj = np.zeros(C, np.float32)
    y = kernel(x=x, W_attn=W_attn, b_attn=b_attn, W_proj=W_proj, b_proj=b_proj)
    print("out", y.shape, y.dtype, np.abs(y).max())
